# revision 3
# baseline (speedup 1.0000x reference)
"""Banded-matmul Trainium2 kernel for nn_AttentionModuleEx1.

Per core (1 sample, data-parallel over batch=8):
  attn0 = K5(x)                   -- 25 taps split PE-diag / DVE lanes
  Z_i   = W_i(attn0)              -- per-channel banded matmuls in L_W layout
  S'    = sum_i H_i(Z_i)          -- banded matmuls in L_H layout, PSUM-accum
  S     = attn0 + S'              -- in-place add
  out   = (W3 S + b3') * x        -- b3' absorbs all bias planes (host-folded)

xbar semantics (verified): dma_start_transpose with out [128,B,C]:
  out[a,b,i] = in[i, b*128 + a]
cm (h,w)-major --xbar--> L_W [p=(h&1, w), f=(h>>1, c)]; band lhsT [64,64]
per (channel, conv) at partitions 0:64, DVE-duplicated to 64:128; each conv
= 2 half-matmuls of 32 rows. Z copies pick f=(hh,c) so Z-xbar returns plain
(h,w)-major cm; engine repack to (w,h)-major; xbar -> L_H [p=(w&1,h)];
H-stage accumulates 3 branches in PSUM; exit xbar is (w,h)-major cm.
"""

import os
import sys

for p in ("/opt/trn_rl_repo", "/opt/pypackages"):
    if p not in sys.path:
        sys.path.insert(0, p)

import numpy as np

C, H, W = 512, 64, 64
NBLK, P = 4, 128
KS = (7, 11, 21)
RS = (9, 15, 30)
N_PE5 = int(os.environ.get("N_PE5", "17"))
_order = [0, 24, 12, 6, 18, 2, 22, 10, 14, 4, 20, 8, 16, 1, 23, 11, 13, 3, 21,
          7, 17, 5, 19, 9, 15]
PE_TAPS = sorted(_order[:N_PE5])
DVE_TAPS = sorted(_order[N_PE5:])

_NC = None


def _build_nc():
    import concourse.bass as bass  # noqa: F401
    import concourse.bacc as bacc
    import concourse.mybir as mybir
    from concourse.tile import TileContext

    f32 = mybir.dt.float32
    f16 = mybir.dt.float16
    A = mybir.AluOpType
    AF = mybir.ActivationFunctionType

    nc = bacc.Bacc(None, target_bir_lowering=False)

    xcan_d = nc.dram_tensor("xcan", [C, 68, 68], f16, kind="ExternalInput")
    wd5_d = (nc.dram_tensor("wd5", [NBLK, P, len(PE_TAPS) * P], f16,
                            kind="ExternalInput") if PE_TAPS else None)
    w0t_d = nc.dram_tensor("w0t", [C, 25], f32, kind="ExternalInput")
    bw_d = nc.dram_tensor("bw", [NBLK, 3, 64, P * 64], f16, kind="ExternalInput")
    bh_d = nc.dram_tensor("bh", [NBLK, 8, 64, 3 * 16 * 64], f16, kind="ExternalInput")
    w3_d = nc.dram_tensor("w3", [C, C], f16, kind="ExternalInput")  # W3^T
    b3p_d = nc.dram_tensor("b3p", [C, 1], f32, kind="ExternalInput")
    out_d = nc.dram_tensor("out", [C, H, W], f16, kind="ExternalOutput")

    ENG = {"s": nc.scalar, "v": nc.vector, "g": nc.gpsimd}

    def ecopy(k, out, in_):
        if k == "s":
            nc.scalar.copy(out, in_)
        else:
            ENG[k].tensor_copy(out, in_)
    ZT_ENG = os.environ.get("ZT_ENG", "ssv")       # zt copies per branch
    SP_ENG = os.environ.get("SP_ENG", "sv")        # spt copies by g parity
    RP_ENG = os.environ.get("RP_ENG", "ggg")       # repack per branch
    MG_ENG = os.environ.get("MG_ENG", "g")         # 5x5 psum merge

    with TileContext(nc) as tc:
        with tc.tile_pool(name="res", bufs=1) as MP, \
             tc.tile_pool(name="tmp", bufs=1) as TP, \
             tc.tile_pool(name="band", bufs=1) as BP, \
             tc.tile_pool(name="zc", bufs=2) as ZP, \
             tc.tile_pool(name="io", bufs=1) as IOP, \
             tc.tile_pool(name="ps5", bufs=2, space="PSUM") as PP5, \
             tc.tile_pool(name="psw", bufs=2, space="PSUM") as PPW, \
             tc.tile_pool(name="psh", bufs=2, space="PSUM") as PPH, \
             tc.tile_pool(name="psm", bufs=2, space="PSUM") as PPM:

            attn0 = [MP.tile([P, H, W], f16, tag=f"attn{b}", name=f"attn{b}")
                     for b in range(NBLK)]
            w3ts = []
            for kk in range(NBLK):
                row = []
                for m in range(NBLK):
                    t = MP.tile([P, P], f16, tag=f"w3t{kk}{m}", name=f"w3t{kk}{m}")
                    nc.sync.dma_start(
                        t[:, :], w3_d[kk * P:(kk + 1) * P, m * P:(m + 1) * P])
                    row.append(t)
                w3ts.append(row)
            w0ts, b3ps, xcans = [], [], []
            for b in range(NBLK):
                sl = slice(b * P, (b + 1) * P)
                t = MP.tile([P, 25], f32, tag=f"w0t{b}", name=f"w0t{b}")
                nc.sync.dma_start(t[:, :], w0t_d[sl, :])
                w0ts.append(t)
                t = MP.tile([P, 1], f32, tag=f"b3p{b}", name=f"b3p{b}")
                nc.sync.dma_start(t[:, :], b3p_d[sl, :])
                b3ps.append(t)
                xc = MP.tile([P, 68, 68], f16, tag=f"xcan{b}", name=f"xcan{b}")
                nc.sync.dma_start(xc[:, :, :], xcan_d[sl, :, :])
                xcans.append(xc)

            # ============ stage 1: 5x5 -> attn0 (cm (h,w)-major) =============
            for b in range(NBLK):
                xc = xcans[b]
                a3 = attn0[b]
                dtmp = TP.tile([P, H, W], f16, tag="dtmp", bufs=1, name="dtmp")
                for j, t in enumerate(DVE_TAPS):
                    dh, dw = t // 5, t % 5
                    win = xc[:, dh:dh + 64, dw:dw + 64]
                    if j == 0:
                        nc.vector.tensor_scalar_mul(a3[:, :, :], win,
                                                    w0ts[b][:, t:t + 1])
                    else:
                        nc.vector.tensor_scalar_mul(dtmp[:, :, :], win,
                                                    w0ts[b][:, t:t + 1])
                        nc.vector.tensor_tensor(a3[:, :, :], dtmp[:, :, :],
                                                a3[:, :, :], op=A.add)
                if PE_TAPS:
                    dstk = TP.tile([P, len(PE_TAPS) * P], f16, tag="dstk",
                                   bufs=1, name="dstk")
                    nc.sync.dma_start(dstk[:, :], wd5_d[b])
                    for ch in range(8):
                        ps = PP5.tile([P, 512], f32, tag="ps5", name="ps5")
                        for j, t in enumerate(PE_TAPS):
                            dh, dw = t // 5, t % 5
                            rv = xc[:, dh + 8 * ch:dh + 8 * ch + 8, dw:dw + 64]
                            nc.tensor.matmul(ps[:, :],
                                             dstk[:, j * P:(j + 1) * P], rv,
                                             start=(j == 0),
                                             stop=(j == len(PE_TAPS) - 1))
                        av = a3[:, 8 * ch:8 * ch + 8, :]
                        ps3 = ps.rearrange("p (a b) -> p a b", a=8)
                        if not DVE_TAPS:
                            ecopy("s", av, ps3)
                        elif MG_ENG == "v":
                            nc.vector.tensor_tensor(av, ps3, av, op=A.add)
                        else:
                            mtmp = TP.tile([P, 8, 64], f16, tag="mtmp", bufs=2,
                                           name="mtmp")
                            ecopy("s", mtmp[:, :, :], ps3)
                            nc.gpsimd.tensor_tensor(av, mtmp[:, :, :], av,
                                                    op=A.add)

            # lw-xbars issued early (ACT queue) so blocks pipeline
            lws = []
            for b in range(NBLK):
                lw = ZP.tile([P, 32, P], f16, tag="lw", name="lw")
                nc.scalar.dma_start_transpose(
                    lw[:, :, :], attn0[b][:, :, :].rearrange("p a b -> p (a b)"))
                lws.append(lw)

            # ============ stages 2-7 per block ===============================
            for b in range(NBLK):
                lw = lws[b]
                lhs = []
                for i in range(3):
                    zt = ZP.tile([P, 32, P], f16, tag="zt", name="zt")
                    bwt = BP.tile([P, P * 64], f16, tag="bwt", bufs=1,
                                  name="bwt")
                    nc.sync.dma_start(bwt[0:64, :], bw_d[b, i])
                    nc.vector.tensor_copy(bwt[64:128, :], bwt[0:64, :])
                    for g in range(8):
                        ps = PPW.tile([P, 512], f32, tag="psw", name="psw")
                        for cc in range(16):
                            c = 16 * g + cc
                            bt = bwt[:, 64 * c:64 * c + 64]
                            for h1 in range(2):
                                o = 64 * h1
                                nc.tensor.matmul(
                                    ps[o:o + 64, 32 * cc:32 * cc + 32],
                                    bt[o:o + 64, :], lw[o:o + 64, :, c],
                                    start=True, stop=True)
                        ps3 = ps.rearrange("p (c hh) -> p c hh", c=16)
                        ecopy(ZT_ENG[i],
                              zt[:, :, 16 * g:16 * g + 16]
                              .rearrange("p a b -> p b a"), ps3)
                    zcm = ZP.tile([P, H * W], f16, tag="c64", name="zcm")
                    nc.sync.dma_start_transpose(
                        zcm[:, :].rearrange("p (a b) -> p a b", a=32),
                        zt[:, :, :].rearrange("p a b -> p (a b)"))
                    zwh = ZP.tile([P, W * H], f16, tag="zwh", bufs=1, name="zwh")
                    ecopy(RP_ENG[i],
                          zwh[:, :].rearrange("p (w h) -> p w h", w=W),
                          zcm[:, :].rearrange("p (h w) -> p w h", h=H))
                    lh = ZP.tile([P, 32, P], f16, tag=f"lh{i}", bufs=1,
                                 name=f"lh{i}")
                    nc.sync.dma_start_transpose(lh[:, :, :], zwh[:, :])
                    lhs.append(lh)

                spt = ZP.tile([P, 32, P], f16, tag="spt", bufs=1, name="spt")
                for g in range(8):
                    bht = BP.tile([P, 3 * 16 * 64], f16, tag="bht", bufs=2,
                                  name="bht")
                    nc.sync.dma_start(bht[0:64, :], bh_d[b, g])
                    nc.vector.tensor_copy(bht[64:128, :], bht[0:64, :])
                    ps = PPH.tile([P, 512], f32, tag="psh", name="psh")
                    for cc in range(16):
                        c = 16 * g + cc
                        for i in range(3):
                            bt = bht[:, (i * 16 + cc) * 64:(i * 16 + cc) * 64 + 64]
                            for w1 in range(2):
                                o = 64 * w1
                                nc.tensor.matmul(
                                    ps[o:o + 64, 32 * cc:32 * cc + 32],
                                    bt[o:o + 64, :], lhs[i][o:o + 64, :, c],
                                    start=(i == 0), stop=(i == 2))
                    ps3 = ps.rearrange("p (c whi) -> p c whi", c=16)
                    ecopy(SP_ENG[g % len(SP_ENG)],
                          spt[:, :, 16 * g:16 * g + 16]
                          .rearrange("p a b -> p b a"), ps3)
                spcm = ZP.tile([P, W * H], f16, tag="c64", name="spcm")
                nc.scalar.dma_start_transpose(
                    spcm[:, :].rearrange("p (a b) -> p a b", a=32),
                    spt[:, :, :].rearrange("p a b -> p (a b)"))
                nc.vector.tensor_tensor(
                    attn0[b][:, :, :],
                    spcm[:, :].rearrange("p (w h) -> p h w", w=W),
                    attn0[b][:, :, :], op=A.add)

            # ============ 1x1 conv + bias + mult-by-x ========================
            for m in range(NBLK):
                sl = slice(m * P, (m + 1) * P)
                for nch in range(8):
                    ps = PPM.tile([P, 512], f32, tag="psm", name="psm")
                    for kk in range(NBLK):
                        rv = attn0[kk][:, 8 * nch:8 * nch + 8, :]
                        nc.tensor.matmul(ps[:, :], w3ts[kk][m][:, :], rv,
                                         start=(kk == 0), stop=(kk == NBLK - 1))
                    yb = IOP.tile([P, 8, 64], f16, tag="yb", name="yb")
                    ps3 = ps.rearrange("p (a b) -> p a b", a=8)
                    nc.scalar.activation(yb[:, :, :], ps3, AF.Identity,
                                         bias=b3ps[m][:, 0:1], scale=1.0)
                    ost = IOP.tile([P, 8, 64], f16, tag="ost", name="ost")
                    xv = xcans[m][:, 2 + 8 * nch:2 + 8 * nch + 8, 2:66]
                    nc.vector.tensor_tensor(ost[:, :, :], yb[:, :, :], xv,
                                            op=A.mult)
                    nc.sync.dma_start(out_d[sl, 8 * nch:8 * nch + 8, :],
                                      ost[:, :, :])

    if not nc.is_finalized():
        nc.finalize()
    return nc


def _get_nc():
    global _NC
    if _NC is None:
        _NC = _build_nc()
    return _NC


def _band_stack(wk, k, R):
    """wk: (C, k) f32 -> (NBLK, 64, 128*64) f16 band stack.

    stack[b, p_in, c*64 + p_out] = wk[b*128+c, t]  iff  p_in == p_out + 3t - R
    (out[w'] = sum_t w[t] * in[w' + 3t - R], zero-pad clipped at edges).
    """
    out = np.zeros((NBLK, 64, P, 64), np.float16)
    for t in range(k):
        d = 3 * t - R
        lo, hi = max(0, -d), min(64, 64 - d)
        po = np.arange(lo, hi)
        pi = po + d
        wblk = wk[:, t].reshape(NBLK, P).astype(np.float16)
        for b in range(NBLK):
            out[b, pi, :, po] = wblk[b][None, :]
    return np.ascontiguousarray(out.reshape(NBLK, 64, P * 64))


def _bw_layout(stacks):
    """3 x (NBLK, 64, 128*64) -> (NBLK, 3, 2, 64, 64*64) halves of 64ch."""
    out = np.zeros((NBLK, 3, 2, 64, 64 * 64), np.float16)
    for i, st in enumerate(stacks):
        s = st.reshape(NBLK, 64, P, 64)
        out[:, i, 0] = s[:, :, 0:64].reshape(NBLK, 64, -1)
        out[:, i, 1] = s[:, :, 64:128].reshape(NBLK, 64, -1)
    return np.ascontiguousarray(out)


def _bh_layout(stacks):
    """3 x (NBLK, 64, 128*64) -> (NBLK, 8, 64, 3*16*64) grouped by 16ch."""
    out = np.zeros((NBLK, 8, 64, 3, 16, 64), np.float16)
    for i, st in enumerate(stacks):
        s = st.reshape(NBLK, 64, P, 64)
        for g in range(8):
            out[:, g, :, i] = s[:, :, 16 * g:16 * g + 16]
    return np.ascontiguousarray(out.reshape(NBLK, 8, 64, 3 * 16 * 64))


def _prep_inputs(inputs):
    f32a = lambda a: np.asarray(a, dtype=np.float32)
    g = lambda nm, k: f32a(inputs[nm]).reshape(C, k)

    x = f32a(inputs["x"]).astype(np.float16)
    xcan = np.zeros((x.shape[0], C, 68, 68), np.float16)
    xcan[:, :, 2:66, 2:66] = x

    w0 = g("w0", 25)
    ww = [g("w0_1", 7), g("w1_1", 11), g("w2_1", 21)]
    wh = [g("w0_2", 7), g("w1_2", 11), g("w2_2", 21)]
    bwb = [f32a(inputs["b0_1"]).reshape(C), f32a(inputs["b1_1"]).reshape(C),
           f32a(inputs["b2_1"]).reshape(C)]
    bhb = [f32a(inputs["b0_2"]).reshape(C), f32a(inputs["b1_2"]).reshape(C),
           f32a(inputs["b2_2"]).reshape(C)]
    b0 = f32a(inputs["b0"]).reshape(C)
    b3 = f32a(inputs["b3"]).reshape(C)
    W3 = f32a(inputs["w3"]).reshape(C, C)

    delta = b0.copy()
    for i in range(3):
        sw = ww[i].sum(1)
        sh = wh[i].sum(1)
        delta += b0 * sw * sh + bwb[i] * sh + bhb[i]
    b3p = (b3 + W3 @ delta).astype(np.float32).reshape(C, 1)

    com = {
        "w0t": np.ascontiguousarray(w0.astype(np.float32)),
        "bw": np.ascontiguousarray(np.stack(
            [_band_stack(ww[i], KS[i], RS[i]) for i in range(3)], axis=1)),
        "bh": _bh_layout([_band_stack(wh[i], KS[i], RS[i])
                          for i in range(3)]),
        "w3": np.ascontiguousarray(W3.T.astype(np.float16)),
        "b3p": b3p,
    }
    if PE_TAPS:
        d = np.zeros((NBLK, P, len(PE_TAPS), P), np.float16)
        idx = np.arange(P)
        for j, t in enumerate(PE_TAPS):
            wb = w0[:, t].reshape(NBLK, P).astype(np.float16)
            d[:, idx, j, idx] = wb
        com["wd5"] = np.ascontiguousarray(d.reshape(NBLK, P, len(PE_TAPS) * P))
    return [dict(com, xcan=np.ascontiguousarray(xcan[i]))
            for i in range(x.shape[0])]


def run(inputs, trace=False):
    from concourse.bass_utils import run_bass_kernel_spmd
    nc = _get_nc()
    in_maps = _prep_inputs(inputs)
    res = run_bass_kernel_spmd(nc, in_maps, core_ids=list(range(len(in_maps))),
                               trace=trace)
    out = np.stack([r["out"] for r in res.results], axis=0).astype(np.float32)
    return out, res


def kernel(**inputs):
    out, _ = run(inputs, trace=False)
    return out
